# revision 1
# baseline (speedup 1.0000x reference)
"""Trainium2 Bass kernel for multi-head attention.

Problem: B=4, H=16, S=2048, D=128, fp32.
  scores = (q @ k^T) / sqrt(128); probs = softmax(scores, -1); out = probs @ v

Sharding: 64 (b,h) pairs -> 8 cores x 8 pairs. Fully independent per pair.

Per-(b,h) layout on device (everything "transposed", T-layout):
  qT, kT: [D=128, S=2048] in SBUF (host pre-transposes; contraction dim d on
  partitions).  For each t-tile (128 keys):
    scoresT[t, s] = kT[:, t-tile].T @ qT          (PE, fp32r)
    expT = exp(scoresT / sqrt(D))                 (ACT, fused scale, psum->sbuf)
    outT[d, s]  += v_tile[t, d].T-contraction     (PE: lhsT=v_tile, rhs=expT)
    sums[*, s]  += ones.T @ expT                  (PE: all 128 rows identical)
  then out = outT * reciprocal(sums)              (DVE approx recip + mul)
Host transposes outT -> out per pair.

s is processed in halves of 1024 so PSUM fits exactly in 8 banks:
  scores ping/pong (2x2 banks) + outT acc (2) + sums acc (2).
"""

import sys

sys.path.insert(0, "/opt/trn_rl_repo")

import numpy as np

B, H, S, D = 4, 16, 2048, 128
N_CORES = 8
BH = B * H                      # 64 pairs
BH_PER_CORE = BH // N_CORES     # 8
T_TILES = S // 128              # 16
S_HALF = S // 2                 # 1024
SCALE = float(D) ** -0.5

_cache = {}


def _build_program():
    import concourse.tile as tile
    from concourse import bacc, mybir

    F32 = mybir.dt.float32
    F32R = mybir.dt.float32r

    nc = bacc.Bacc("TRN2", target_bir_lowering=False, debug=False)

    qt = nc.dram_tensor("qt", [BH_PER_CORE, D, S], F32, kind="ExternalInput")
    kt = nc.dram_tensor("kt", [BH_PER_CORE, D, S], F32, kind="ExternalInput")
    v = nc.dram_tensor("v", [BH_PER_CORE, S, D], F32, kind="ExternalInput")
    ot = nc.dram_tensor("ot", [BH_PER_CORE, D, S], F32, kind="ExternalOutput")

    with tile.TileContext(nc) as tc:
        with (
            tc.tile_pool(name="const", bufs=1) as const,
            tc.tile_pool(name="stage", bufs=2) as stage,
            tc.tile_pool(name="rin", bufs=2) as rin,
            tc.tile_pool(name="exps", bufs=3) as exps,
            tc.tile_pool(name="outs", bufs=3) as outs,
            tc.tile_pool(name="psc", bufs=2, space="PSUM") as psc,
            tc.tile_pool(name="pacc", bufs=1, space="PSUM") as pacc,
            tc.tile_pool(name="psum_s", bufs=1, space="PSUM") as psum_s,
        ):
            ones_f = const.tile([128, 128], F32)
            nc.vector.memset(ones_f[:], 1.0)
            ones_r = const.tile([128, 128], F32R)
            nc.vector.tensor_copy(ones_r[:], ones_f[:])

            for i in range(BH_PER_CORE):
                # Load + round inputs for this (b,h) pair.
                q_sb = stage.tile([D, S], F32, tag="q_sb")
                k_sb = stage.tile([D, S], F32, tag="k_sb")
                v_sb = stage.tile([128, T_TILES, D], F32, tag="v_sb")
                nc.sync.dma_start(out=q_sb[:], in_=qt[i])
                nc.sync.dma_start(out=k_sb[:], in_=kt[i])
                nc.sync.dma_start(
                    out=v_sb[:], in_=v[i].rearrange("(t p) d -> p t d", p=128)
                )
                q_r = rin.tile([D, S], F32R, tag="q_r")
                k_r = rin.tile([D, S], F32R, tag="k_r")
                v_r = rin.tile([128, T_TILES, D], F32R, tag="v_r")
                nc.vector.tensor_copy(q_r[:], q_sb[:])
                nc.vector.tensor_copy(k_r[:], k_sb[:])
                nc.vector.tensor_copy(v_r[:], v_sb[:])

                for h in range(2):
                    s0 = h * S_HALF
                    oacc = pacc.tile([128, S_HALF], F32, tag="oacc")
                    sacc = psum_s.tile([128, S_HALF], F32, tag="sacc")

                    for t in range(T_TILES):
                        sc = psc.tile([128, S_HALF], F32, tag="sc")
                        for c in range(0, S_HALF, 512):
                            nc.tensor.matmul(
                                sc[:, c : c + 512],
                                k_r[:, t * 128 : (t + 1) * 128],
                                q_r[:, s0 + c : s0 + c + 512],
                                start=True,
                                stop=True,
                            )
                        et = exps.tile([128, S_HALF], F32R, tag="et")
                        nc.scalar.activation(
                            et[:],
                            sc[:],
                            mybir.ActivationFunctionType.Exp,
                            scale=SCALE,
                        )
                        for c in range(0, S_HALF, 512):
                            nc.tensor.matmul(
                                oacc[:, c : c + 512],
                                v_r[:, t, :],
                                et[:, c : c + 512],
                                start=(t == 0),
                                stop=(t == T_TILES - 1),
                            )
                        for c in range(0, S_HALF, 512):
                            nc.tensor.matmul(
                                sacc[:, c : c + 512],
                                ones_r[:],
                                et[:, c : c + 512],
                                start=(t == 0),
                                stop=(t == T_TILES - 1),
                            )

                    rec = outs.tile([128, S_HALF], F32, tag="rec")
                    nc.vector.reciprocal_approx_fast(out=rec[:], in_=sacc[:])
                    osb = outs.tile([128, S_HALF], F32, tag="osb")
                    nc.vector.tensor_mul(osb[:], oacc[:], rec[:])
                    nc.sync.dma_start(out=ot[i, :, s0 : s0 + S_HALF], in_=osb[:])

    nc.finalize()
    return nc


def _get_program():
    if "nc" not in _cache:
        _cache["nc"] = _build_program()
    return _cache["nc"]


def kernel(q: np.ndarray, k: np.ndarray, v: np.ndarray) -> np.ndarray:
    from concourse.bass_utils import run_bass_kernel_spmd

    nc = _get_program()

    q4 = np.ascontiguousarray(q, dtype=np.float32).reshape(BH, S, D)
    k4 = np.ascontiguousarray(k, dtype=np.float32).reshape(BH, S, D)
    v4 = np.ascontiguousarray(v, dtype=np.float32).reshape(BH, S, D)

    in_maps = []
    for core in range(N_CORES):
        sl = slice(core * BH_PER_CORE, (core + 1) * BH_PER_CORE)
        in_maps.append(
            {
                "qt": np.ascontiguousarray(q4[sl].transpose(0, 2, 1)),
                "kt": np.ascontiguousarray(k4[sl].transpose(0, 2, 1)),
                "v": np.ascontiguousarray(v4[sl]),
            }
        )

    res = run_bass_kernel_spmd(nc, in_maps, core_ids=list(range(N_CORES)))

    out = np.empty((BH, S, D), dtype=np.float32)
    for core in range(N_CORES):
        ot = res.results[core]["ot"]  # [BH_PER_CORE, D, S]
        out[core * BH_PER_CORE : (core + 1) * BH_PER_CORE] = ot.transpose(0, 2, 1)
    return out.reshape(B, H, S, D)


# revision 4
# speedup vs baseline: 1.2460x; 1.2460x over previous
"""Trainium2 Bass kernel for multi-head attention.

Problem: B=4, H=16, S=2048, D=128, fp32.
  scores = (q @ k^T) / sqrt(128); probs = softmax(scores, -1); out = probs @ v

Sharding: 64 (b,h) pairs -> 8 cores x 8 pairs. Fully independent per pair.

Per-(b,h) layout on device (everything "transposed", T-layout):
  qT, kT: [D=128, S=2048] in SBUF (host pre-transposes; contraction dim d on
  partitions).  For each t-tile (128 keys):
    scoresT[t, s] = kT[:, t-tile].T @ qT          (PE, fp32r)
    expT = exp(scoresT / sqrt(D))                 (ACT, fused scale, psum->sbuf)
    outT[d, s]  += v_tile[t, d].T-contraction     (PE: lhsT=v_tile, rhs=expT)
    sums[*, s]  += ones.T @ expT                  (PE: all 128 rows identical)
  then out = outT * reciprocal(sums)              (DVE approx recip + mul)
Host transposes outT -> out per pair.

s is processed in halves of 1024 so PSUM fits exactly in 8 banks:
  scores ping/pong (2x2 banks) + outT acc (2) + sums acc (2).
"""

import sys

sys.path.insert(0, "/opt/trn_rl_repo")

import numpy as np

B, H, S, D = 4, 16, 2048, 128
N_CORES = 8
BH = B * H                      # 64 pairs
BH_PER_CORE = BH // N_CORES     # 8
T_TILES = S // 128              # 16
S_HALF = S // 2                 # 1024
SCALE = float(D) ** -0.5

_cache = {}


def _build_program():
    import concourse.tile as tile
    from concourse import bacc, mybir

    F32 = mybir.dt.float32
    F32R = mybir.dt.float32r

    nc = bacc.Bacc("TRN2", target_bir_lowering=False, debug=False)

    qt = nc.dram_tensor("qt", [BH_PER_CORE, D, S], F32, kind="ExternalInput")
    kt = nc.dram_tensor("kt", [BH_PER_CORE, D, S], F32, kind="ExternalInput")
    v = nc.dram_tensor("v", [BH_PER_CORE, S, D], F32, kind="ExternalInput")
    ot = nc.dram_tensor("ot", [BH_PER_CORE, D, S], F32, kind="ExternalOutput")

    with tile.TileContext(nc) as tc:
        with (
            tc.tile_pool(name="const", bufs=1) as const,
            tc.tile_pool(name="stage", bufs=2) as stage,
            tc.tile_pool(name="rin", bufs=2) as rin,
            tc.tile_pool(name="exps", bufs=4) as exps,
            tc.tile_pool(name="outs", bufs=3) as outs,
            tc.tile_pool(name="psc", bufs=2, space="PSUM") as psc,
            tc.tile_pool(name="pacc", bufs=1, space="PSUM") as pacc,
            tc.tile_pool(name="psum_s", bufs=1, space="PSUM") as psum_s,
        ):
            ones_f = const.tile([128, 128], F32)
            nc.vector.memset(ones_f[:], 1.0)
            ones_r = const.tile([128, 128], F32R)
            nc.vector.tensor_copy(ones_r[:], ones_f[:])

            for i in range(BH_PER_CORE):
                # Load + round inputs for this (b,h) pair.
                q_sb = stage.tile([D, S], F32, tag="q_sb")
                k_sb = stage.tile([D, S], F32, tag="k_sb")
                v_sb = stage.tile([128, T_TILES, D], F32, tag="v_sb")
                nc.sync.dma_start(out=q_sb[:], in_=qt[i])
                nc.sync.dma_start(out=k_sb[:], in_=kt[i])
                nc.sync.dma_start(
                    out=v_sb[:], in_=v[i].rearrange("(t p) d -> p t d", p=128)
                )
                q_r = rin.tile([D, S], F32R, tag="q_r")
                k_r = rin.tile([D, S], F32R, tag="k_r")
                v_r = rin.tile([128, T_TILES, D], F32R, tag="v_r")
                nc.vector.tensor_copy(q_r[:], q_sb[:])
                nc.vector.tensor_copy(k_r[:], k_sb[:])
                nc.vector.tensor_copy(v_r[:], v_sb[:])

                for h in range(2):
                    s0 = h * S_HALF
                    oacc = pacc.tile([128, S_HALF], F32, tag="oacc")
                    sacc = psum_s.tile([128, S_HALF], F32, tag="sacc")

                    # Software-pipelined by one tile: issue scores+exp for
                    # tile t, then the exp-consuming matmuls for tile t-1,
                    # so the PE never waits on ACT's exp latency.
                    ets = [None] * T_TILES

                    def consume(t):
                        for c in range(0, S_HALF, 512):
                            nc.tensor.matmul(
                                oacc[:, c : c + 512],
                                v_r[:, t, :],
                                ets[t][:, c : c + 512],
                                start=(t == 0),
                                stop=(t == T_TILES - 1),
                            )
                        for c in range(0, S_HALF, 512):
                            nc.tensor.matmul(
                                sacc[:, c : c + 512],
                                ones_r[:],
                                ets[t][:, c : c + 512],
                                start=(t == 0),
                                stop=(t == T_TILES - 1),
                            )

                    for t in range(T_TILES):
                        sc = psc.tile([128, S_HALF], F32, tag="sc")
                        for c in range(0, S_HALF, 512):
                            nc.tensor.matmul(
                                sc[:, c : c + 512],
                                k_r[:, t * 128 : (t + 1) * 128],
                                q_r[:, s0 + c : s0 + c + 512],
                                start=True,
                                stop=True,
                            )
                        ets[t] = exps.tile([128, S_HALF], F32R, tag="et", name=f"et_{t}")
                        nc.scalar.activation(
                            ets[t][:],
                            sc[:],
                            mybir.ActivationFunctionType.Exp,
                            scale=SCALE,
                        )
                        if t >= 1:
                            consume(t - 1)
                    consume(T_TILES - 1)

                    rec = outs.tile([128, S_HALF], F32, tag="rec")
                    nc.vector.reciprocal_approx_fast(out=rec[:], in_=sacc[:])
                    osb = outs.tile([128, S_HALF], F32, tag="osb")
                    nc.vector.tensor_mul(osb[:], oacc[:], rec[:])
                    nc.sync.dma_start(out=ot[i, :, s0 : s0 + S_HALF], in_=osb[:])

    nc.finalize()
    return nc


def _get_program():
    if "nc" not in _cache:
        _cache["nc"] = _build_program()
    return _cache["nc"]


def kernel(q: np.ndarray, k: np.ndarray, v: np.ndarray) -> np.ndarray:
    from concourse.bass_utils import run_bass_kernel_spmd

    nc = _get_program()

    q4 = np.ascontiguousarray(q, dtype=np.float32).reshape(BH, S, D)
    k4 = np.ascontiguousarray(k, dtype=np.float32).reshape(BH, S, D)
    v4 = np.ascontiguousarray(v, dtype=np.float32).reshape(BH, S, D)

    in_maps = []
    for core in range(N_CORES):
        sl = slice(core * BH_PER_CORE, (core + 1) * BH_PER_CORE)
        in_maps.append(
            {
                "qt": np.ascontiguousarray(q4[sl].transpose(0, 2, 1)),
                "kt": np.ascontiguousarray(k4[sl].transpose(0, 2, 1)),
                "v": np.ascontiguousarray(v4[sl]),
            }
        )

    res = run_bass_kernel_spmd(nc, in_maps, core_ids=list(range(N_CORES)))

    out = np.empty((BH, S, D), dtype=np.float32)
    for core in range(N_CORES):
        ot = res.results[core]["ot"]  # [BH_PER_CORE, D, S]
        out[core * BH_PER_CORE : (core + 1) * BH_PER_CORE] = ot.transpose(0, 2, 1)
    return out.reshape(B, H, S, D)


# revision 7
# speedup vs baseline: 1.3407x; 1.0760x over previous
"""Trainium2 Bass kernel for multi-head attention.

Problem: B=4, H=16, S=2048, D=128, fp32.
  scores = (q @ k^T) / sqrt(128); probs = softmax(scores, -1); out = probs @ v

Sharding: 64 (b,h) pairs -> 8 cores x 8 pairs. Fully independent per pair.

Per-(b,h) layout on device (everything "transposed", T-layout):
  qT, kT: [D=128, S=2048] in SBUF (host pre-transposes; contraction dim d on
  partitions).  For each t-tile (128 keys):
    scoresT[t, s] = kT[:, t-tile].T @ qT          (PE, fp32r)
    expT = exp(scoresT / sqrt(D))                 (ACT, fused scale, psum->sbuf)
    outT[d, s]  += v_tile[t, d].T-contraction     (PE: lhsT=v_tile, rhs=expT)
  softmax denominators (partition-dim sum of expT):
    tiles 0..3:  sums += ones.T @ expT            (PE; output rows replicated)
    tiles 4..15: acc += expT elementwise (DVE),   then sums += ones.T @ acc
  then out = outT * reciprocal(sums)              (DVE approx recip + mul)
Host transposes outT -> out per pair.

s is processed in halves of 1024 so PSUM fits exactly in 8 banks:
  scores ping/pong (2x2 banks) + outT acc (2) + sums acc (2).
The per-tile loop is software-pipelined by one tile so the PE never waits
on ACT's exp latency.
"""

import sys

sys.path.insert(0, "/opt/trn_rl_repo")

import numpy as np

B, H, S, D = 4, 16, 2048, 128
N_CORES = 8
BH = B * H                      # 64 pairs
BH_PER_CORE = BH // N_CORES     # 8
T_TILES = S // 128              # 16
S_HALF = S // 2                 # 1024
SCALE = float(D) ** -0.5
PE_SUM_TILES = 4                # tiles summed via PE ones-matmul directly

_cache = {}


def _build_program():
    import concourse.tile as tile
    from concourse import bacc, mybir

    F32 = mybir.dt.float32
    F32R = mybir.dt.float32r

    nc = bacc.Bacc("TRN2", target_bir_lowering=False, debug=False)

    qt = nc.dram_tensor("qt", [BH_PER_CORE, D, S], F32R, kind="ExternalInput")
    kt = nc.dram_tensor("kt", [BH_PER_CORE, D, S], F32R, kind="ExternalInput")
    v = nc.dram_tensor("v", [BH_PER_CORE, S, D], F32R, kind="ExternalInput")
    ot = nc.dram_tensor("ot", [BH_PER_CORE, D, S], F32, kind="ExternalOutput")

    with tile.TileContext(nc) as tc:
        with (
            tc.tile_pool(name="const", bufs=1) as const,
            tc.tile_pool(name="rin", bufs=2) as rin,
            tc.tile_pool(name="exps", bufs=7) as exps,
            tc.tile_pool(name="accp", bufs=2) as accp,
            tc.tile_pool(name="outs", bufs=3) as outs,
            tc.tile_pool(name="psc", bufs=2, space="PSUM") as psc,
            tc.tile_pool(name="pacc", bufs=1, space="PSUM") as pacc,
            tc.tile_pool(name="psum_s", bufs=1, space="PSUM") as psum_s,
        ):
            ones_f = const.tile([128, 128], F32)
            nc.vector.memset(ones_f[:], 1.0)
            ones_r = const.tile([128, 128], F32R)
            nc.vector.tensor_copy(ones_r[:], ones_f[:])

            for i in range(BH_PER_CORE):
                q_r = rin.tile([D, S], F32R, tag="q_r")
                k_r = rin.tile([D, S], F32R, tag="k_r")
                v_r = rin.tile([128, T_TILES, D], F32R, tag="v_r")
                nc.sync.dma_start(out=q_r[:], in_=qt[i])
                nc.sync.dma_start(out=k_r[:], in_=kt[i])
                nc.sync.dma_start(
                    out=v_r[:], in_=v[i].rearrange("(t p) d -> p t d", p=128)
                )

                for h in range(2):
                    s0 = h * S_HALF
                    oacc = pacc.tile([128, S_HALF], F32, tag="oacc")
                    sacc = psum_s.tile([128, S_HALF], F32, tag="sacc")
                    eacc = accp.tile([128, S_HALF], F32R, tag="eacc")

                    ets = [None] * T_TILES

                    def pv(t):
                        for c in range(0, S_HALF, 512):
                            nc.tensor.matmul(
                                oacc[:, c : c + 512],
                                v_r[:, t, :],
                                ets[t][:, c : c + 512],
                                start=(t == 0),
                                stop=(t == T_TILES - 1),
                            )

                    def consume(t):
                        pv(t)
                        if t == PE_SUM_TILES - 1:
                            # PE ones-matmuls for tiles 0..PE_SUM_TILES-1
                            for tt in range(PE_SUM_TILES):
                                for c in range(0, S_HALF, 512):
                                    nc.tensor.matmul(
                                        sacc[:, c : c + 512],
                                        ones_r[:],
                                        ets[tt][:, c : c + 512],
                                        start=(tt == 0),
                                        stop=False,
                                    )
                        elif t == PE_SUM_TILES + 1:
                            nc.vector.tensor_add(
                                eacc[:], ets[t - 1][:], ets[t][:]
                            )
                        elif t > PE_SUM_TILES + 1:
                            nc.vector.tensor_add(eacc[:], eacc[:], ets[t][:])

                    for t in range(T_TILES):
                        sc = psc.tile([128, S_HALF], F32, tag="sc")
                        for c in range(0, S_HALF, 512):
                            nc.tensor.matmul(
                                sc[:, c : c + 512],
                                k_r[:, t * 128 : (t + 1) * 128],
                                q_r[:, s0 + c : s0 + c + 512],
                                start=True,
                                stop=True,
                            )
                        ets[t] = exps.tile(
                            [128, S_HALF], F32R, tag="et", name=f"et_{t}"
                        )
                        nc.scalar.activation(
                            ets[t][:],
                            sc[:],
                            mybir.ActivationFunctionType.Exp,
                            scale=SCALE,
                        )
                        if t >= 1:
                            consume(t - 1)
                    consume(T_TILES - 1)

                    # finish the denominator with the DVE accumulator.
                    for c in range(0, S_HALF, 512):
                        nc.tensor.matmul(
                            sacc[:, c : c + 512],
                            ones_r[:],
                            eacc[:, c : c + 512],
                            start=False,
                            stop=True,
                        )

                    rec = outs.tile([128, S_HALF], F32, tag="rec")
                    nc.vector.reciprocal_approx_fast(out=rec[:], in_=sacc[:])
                    osb = outs.tile([128, S_HALF], F32, tag="osb")
                    nc.vector.tensor_mul(osb[:], oacc[:], rec[:])
                    nc.sync.dma_start(out=ot[i, :, s0 : s0 + S_HALF], in_=osb[:])

    nc.finalize()
    return nc


def _get_program():
    if "nc" not in _cache:
        _cache["nc"] = _build_program()
    return _cache["nc"]


def kernel(q: np.ndarray, k: np.ndarray, v: np.ndarray) -> np.ndarray:
    from concourse.bass_utils import run_bass_kernel_spmd

    nc = _get_program()

    q4 = np.ascontiguousarray(q, dtype=np.float32).reshape(BH, S, D)
    k4 = np.ascontiguousarray(k, dtype=np.float32).reshape(BH, S, D)
    v4 = np.ascontiguousarray(v, dtype=np.float32).reshape(BH, S, D)

    in_maps = []
    for core in range(N_CORES):
        sl = slice(core * BH_PER_CORE, (core + 1) * BH_PER_CORE)
        in_maps.append(
            {
                "qt": np.ascontiguousarray(q4[sl].transpose(0, 2, 1)),
                "kt": np.ascontiguousarray(k4[sl].transpose(0, 2, 1)),
                "v": np.ascontiguousarray(v4[sl]),
            }
        )

    res = run_bass_kernel_spmd(nc, in_maps, core_ids=list(range(N_CORES)))

    out = np.empty((BH, S, D), dtype=np.float32)
    for core in range(N_CORES):
        ot = res.results[core]["ot"]  # [BH_PER_CORE, D, S]
        out[core * BH_PER_CORE : (core + 1) * BH_PER_CORE] = ot.transpose(0, 2, 1)
    return out.reshape(B, H, S, D)


# revision 9
# speedup vs baseline: 1.3641x; 1.0175x over previous
"""Trainium2 Bass kernel for multi-head attention.

Problem: B=4, H=16, S=2048, D=128, fp32.
  scores = (q @ k^T) / sqrt(128); probs = softmax(scores, -1); out = probs @ v

Sharding: 64 (b,h) pairs -> 8 cores x 8 pairs. Fully independent per pair.

Per-(b,h) layout on device (everything "transposed", T-layout):
  qT, kT: [D=128, S=2048] in SBUF (host pre-transposes; contraction dim d on
  partitions).  For each t-tile (128 keys):
    scoresT[t, s] = kT[:, t-tile].T @ qT          (PE, fp32r)
    expT = exp(scoresT / sqrt(D))                 (ACT, fused scale, psum->sbuf)
    outT[d, s]  += v_tile[t, d].T-contraction     (PE: lhsT=v_tile, rhs=expT)
  softmax denominators (partition-dim sum of expT):
    tiles 0..3:  sums += ones.T @ expT            (PE; output rows replicated)
    tiles 4..15: acc += expT elementwise (DVE),   then sums += ones.T @ acc
  then out = outT * reciprocal(sums)              (DVE approx recip + mul)
Host transposes outT -> out per pair.

s is processed in halves of 1024 so PSUM fits exactly in 8 banks:
  scores ping/pong (2x2 banks) + outT acc (2) + sums acc (2).
The per-tile loop is software-pipelined by one tile so the PE never waits
on ACT's exp latency.
"""

import sys

sys.path.insert(0, "/opt/trn_rl_repo")

import numpy as np

B, H, S, D = 4, 16, 2048, 128
N_CORES = 8
BH = B * H                      # 64 pairs
BH_PER_CORE = BH // N_CORES     # 8
T_TILES = S // 128              # 16
S_HALF = S // 2                 # 1024
SCALE = float(D) ** -0.5
PE_SUM_TILES = 3                # tiles summed via PE ones-matmul directly

_cache = {}


def _build_program():
    import concourse.tile as tile
    from concourse import bacc, mybir

    F32 = mybir.dt.float32
    F32R = mybir.dt.float32r

    nc = bacc.Bacc("TRN2", target_bir_lowering=False, debug=False)

    qt = nc.dram_tensor("qt", [BH_PER_CORE, D, S], F32R, kind="ExternalInput")
    kt = nc.dram_tensor("kt", [BH_PER_CORE, D, S], F32R, kind="ExternalInput")
    v = nc.dram_tensor("v", [BH_PER_CORE, S, D], F32R, kind="ExternalInput")
    ot = nc.dram_tensor("ot", [BH_PER_CORE, D, S], F32, kind="ExternalOutput")

    with tile.TileContext(nc) as tc:
        with (
            tc.tile_pool(name="const", bufs=1) as const,
            tc.tile_pool(name="rin", bufs=2) as rin,
            tc.tile_pool(name="exps", bufs=7) as exps,
            tc.tile_pool(name="accp", bufs=2) as accp,
            tc.tile_pool(name="outs", bufs=3) as outs,
            tc.tile_pool(name="psc", bufs=2, space="PSUM") as psc,
            tc.tile_pool(name="pacc", bufs=1, space="PSUM") as pacc,
            tc.tile_pool(name="psum_s", bufs=1, space="PSUM") as psum_s,
        ):
            ones_f = const.tile([128, 128], F32)
            nc.vector.memset(ones_f[:], 1.0)
            ones_r = const.tile([128, 128], F32R)
            nc.vector.tensor_copy(ones_r[:], ones_f[:])

            for i in range(BH_PER_CORE):
                q_r = rin.tile([D, S], F32R, tag="q_r")
                k_r = rin.tile([D, S], F32R, tag="k_r")
                v_r = rin.tile([128, T_TILES, D], F32R, tag="v_r")
                nc.sync.dma_start(out=q_r[:], in_=qt[i])
                nc.sync.dma_start(out=k_r[:], in_=kt[i])
                nc.sync.dma_start(
                    out=v_r[:], in_=v[i].rearrange("(t p) d -> p t d", p=128)
                )

                for h in range(2):
                    s0 = h * S_HALF
                    oacc = pacc.tile([128, S_HALF], F32, tag="oacc")
                    sacc = psum_s.tile([128, S_HALF], F32, tag="sacc")
                    eacc = accp.tile([128, S_HALF], F32R, tag="eacc")

                    ets = [None] * T_TILES

                    def pv(t):
                        for c in range(0, S_HALF, 512):
                            nc.tensor.matmul(
                                oacc[:, c : c + 512],
                                v_r[:, t, :],
                                ets[t][:, c : c + 512],
                                start=(t == 0),
                                stop=(t == T_TILES - 1),
                            )

                    def consume(t):
                        pv(t)
                        if t < PE_SUM_TILES:
                            # PE ones-matmuls, one tile at a time
                            for c in range(0, S_HALF, 512):
                                nc.tensor.matmul(
                                    sacc[:, c : c + 512],
                                    ones_r[:],
                                    ets[t][:, c : c + 512],
                                    start=(t == 0),
                                    stop=False,
                                )
                        elif t == PE_SUM_TILES + 1:
                            nc.vector.tensor_add(
                                eacc[:], ets[t - 1][:], ets[t][:]
                            )
                        elif t > PE_SUM_TILES + 1:
                            nc.vector.tensor_add(eacc[:], eacc[:], ets[t][:])

                    for t in range(T_TILES):
                        sc = psc.tile([128, S_HALF], F32, tag="sc")
                        for c in range(0, S_HALF, 512):
                            nc.tensor.matmul(
                                sc[:, c : c + 512],
                                k_r[:, t * 128 : (t + 1) * 128],
                                q_r[:, s0 + c : s0 + c + 512],
                                start=True,
                                stop=True,
                            )
                        ets[t] = exps.tile(
                            [128, S_HALF], F32R, tag="et", name=f"et_{t}"
                        )
                        nc.scalar.activation(
                            ets[t][:],
                            sc[:],
                            mybir.ActivationFunctionType.Exp,
                            scale=SCALE,
                        )
                        if t >= 1:
                            consume(t - 1)
                    consume(T_TILES - 1)

                    # finish the denominator with the DVE accumulator.
                    for c in range(0, S_HALF, 512):
                        nc.tensor.matmul(
                            sacc[:, c : c + 512],
                            ones_r[:],
                            eacc[:, c : c + 512],
                            start=False,
                            stop=True,
                        )

                    rec = outs.tile([128, S_HALF], F32, tag="rec")
                    nc.vector.reciprocal_approx_fast(out=rec[:], in_=sacc[:])
                    osb = outs.tile([128, S_HALF], F32, tag="osb")
                    nc.vector.tensor_mul(osb[:], oacc[:], rec[:])
                    nc.sync.dma_start(out=ot[i, :, s0 : s0 + S_HALF], in_=osb[:])

    nc.finalize()
    return nc


def _get_program():
    if "nc" not in _cache:
        _cache["nc"] = _build_program()
    return _cache["nc"]


def kernel(q: np.ndarray, k: np.ndarray, v: np.ndarray) -> np.ndarray:
    from concourse.bass_utils import run_bass_kernel_spmd

    nc = _get_program()

    q4 = np.ascontiguousarray(q, dtype=np.float32).reshape(BH, S, D)
    k4 = np.ascontiguousarray(k, dtype=np.float32).reshape(BH, S, D)
    v4 = np.ascontiguousarray(v, dtype=np.float32).reshape(BH, S, D)

    in_maps = []
    for core in range(N_CORES):
        sl = slice(core * BH_PER_CORE, (core + 1) * BH_PER_CORE)
        in_maps.append(
            {
                "qt": np.ascontiguousarray(q4[sl].transpose(0, 2, 1)),
                "kt": np.ascontiguousarray(k4[sl].transpose(0, 2, 1)),
                "v": np.ascontiguousarray(v4[sl]),
            }
        )

    res = run_bass_kernel_spmd(nc, in_maps, core_ids=list(range(N_CORES)))

    out = np.empty((BH, S, D), dtype=np.float32)
    for core in range(N_CORES):
        ot = res.results[core]["ot"]  # [BH_PER_CORE, D, S]
        out[core * BH_PER_CORE : (core + 1) * BH_PER_CORE] = ot.transpose(0, 2, 1)
    return out.reshape(B, H, S, D)


# revision 10
# speedup vs baseline: 1.3678x; 1.0027x over previous
"""Trainium2 Bass kernel for multi-head attention.

Problem: B=4, H=16, S=2048, D=128, fp32.
  scores = (q @ k^T) / sqrt(128); probs = softmax(scores, -1); out = probs @ v

Sharding: 64 (b,h) pairs -> 8 cores x 8 pairs. Fully independent per pair.

Per-(b,h) layout on device (everything "transposed", T-layout):
  qT, kT: [D=128, S=2048] in SBUF (host pre-transposes; contraction dim d on
  partitions).  For each t-tile (128 keys):
    scoresT[t, s] = kT[:, t-tile].T @ qT          (PE, fp32r)
    expT = exp(scoresT / sqrt(D))                 (ACT, fused scale, psum->sbuf)
    outT[d, s]  += v_tile[t, d].T-contraction     (PE: lhsT=v_tile, rhs=expT)
  softmax denominators (partition-dim sum of expT):
    tiles 0..3:  sums += ones.T @ expT            (PE; output rows replicated)
    tiles 4..15: acc += expT elementwise (DVE),   then sums += ones.T @ acc
  then out = outT * reciprocal(sums)              (DVE approx recip + mul)
Host transposes outT -> out per pair.

s is processed in halves of 1024 so PSUM fits exactly in 8 banks:
  scores ping/pong (2x2 banks) + outT acc (2) + sums acc (2).
The per-tile loop is software-pipelined by one tile so the PE never waits
on ACT's exp latency.
"""

import sys

sys.path.insert(0, "/opt/trn_rl_repo")

import numpy as np

B, H, S, D = 4, 16, 2048, 128
N_CORES = 8
BH = B * H                      # 64 pairs
BH_PER_CORE = BH // N_CORES     # 8
T_TILES = S // 128              # 16
S_HALF = S // 2                 # 1024
SCALE = float(D) ** -0.5
PE_SUM_TILES = 3                # tiles summed via PE ones-matmul directly

_cache = {}


def _build_program():
    import concourse.tile as tile
    from concourse import bacc, mybir

    F32 = mybir.dt.float32
    F32R = mybir.dt.float32r

    nc = bacc.Bacc("TRN2", target_bir_lowering=False, debug=False)

    qt = nc.dram_tensor("qt", [BH_PER_CORE, D, S], F32R, kind="ExternalInput")
    kt = nc.dram_tensor("kt", [BH_PER_CORE, D, S], F32R, kind="ExternalInput")
    v = nc.dram_tensor("v", [BH_PER_CORE, S, D], F32R, kind="ExternalInput")
    ot = nc.dram_tensor("ot", [BH_PER_CORE, D, S], F32, kind="ExternalOutput")

    with tile.TileContext(nc) as tc:
        with (
            tc.tile_pool(name="const", bufs=1) as const,
            tc.tile_pool(name="rin", bufs=2) as rin,
            tc.tile_pool(name="exps", bufs=12) as exps,
            tc.tile_pool(name="accp", bufs=2) as accp,
            tc.tile_pool(name="outs", bufs=3) as outs,
            tc.tile_pool(name="psc", bufs=2, space="PSUM") as psc,
            tc.tile_pool(name="pacc", bufs=1, space="PSUM") as pacc,
            tc.tile_pool(name="psum_s", bufs=1, space="PSUM") as psum_s,
        ):
            ones_f = const.tile([128, 128], F32)
            nc.vector.memset(ones_f[:], 1.0)
            ones_r = const.tile([128, 128], F32R)
            nc.vector.tensor_copy(ones_r[:], ones_f[:])

            for i in range(BH_PER_CORE):
                q_r = rin.tile([D, S], F32R, tag="q_r")
                k_r = rin.tile([D, S], F32R, tag="k_r")
                v_r = rin.tile([128, T_TILES, D], F32R, tag="v_r")
                nc.sync.dma_start(out=q_r[:], in_=qt[i])
                nc.sync.dma_start(out=k_r[:], in_=kt[i])
                nc.sync.dma_start(
                    out=v_r[:], in_=v[i].rearrange("(t p) d -> p t d", p=128)
                )

                for h in range(2):
                    s0 = h * S_HALF
                    oacc = pacc.tile([128, S_HALF], F32, tag="oacc")
                    sacc = psum_s.tile([128, S_HALF], F32, tag="sacc")
                    eacc = accp.tile([128, S_HALF], F32R, tag="eacc")

                    ets = [None] * T_TILES

                    def pv(t):
                        for c in range(0, S_HALF, 512):
                            nc.tensor.matmul(
                                oacc[:, c : c + 512],
                                v_r[:, t, :],
                                ets[t][:, c : c + 512],
                                start=(t == 0),
                                stop=(t == T_TILES - 1),
                            )

                    def consume(t):
                        pv(t)
                        if t < PE_SUM_TILES:
                            # PE ones-matmuls, one tile at a time
                            for c in range(0, S_HALF, 512):
                                nc.tensor.matmul(
                                    sacc[:, c : c + 512],
                                    ones_r[:],
                                    ets[t][:, c : c + 512],
                                    start=(t == 0),
                                    stop=False,
                                )
                        elif t == PE_SUM_TILES + 1:
                            nc.vector.tensor_add(
                                eacc[:], ets[t - 1][:], ets[t][:]
                            )
                        elif t > PE_SUM_TILES + 1:
                            nc.vector.tensor_add(eacc[:], eacc[:], ets[t][:])

                    for t in range(T_TILES):
                        sc = psc.tile([128, S_HALF], F32, tag="sc")
                        for c in range(0, S_HALF, 512):
                            nc.tensor.matmul(
                                sc[:, c : c + 512],
                                k_r[:, t * 128 : (t + 1) * 128],
                                q_r[:, s0 + c : s0 + c + 512],
                                start=True,
                                stop=True,
                            )
                        ets[t] = exps.tile(
                            [128, S_HALF], F32R, tag="et", name=f"et_{t}"
                        )
                        nc.scalar.activation(
                            ets[t][:],
                            sc[:],
                            mybir.ActivationFunctionType.Exp,
                            scale=SCALE,
                        )
                        if t >= 1:
                            consume(t - 1)
                    consume(T_TILES - 1)

                    # finish the denominator with the DVE accumulator.
                    for c in range(0, S_HALF, 512):
                        nc.tensor.matmul(
                            sacc[:, c : c + 512],
                            ones_r[:],
                            eacc[:, c : c + 512],
                            start=False,
                            stop=True,
                        )

                    rec = outs.tile([128, S_HALF], F32, tag="rec")
                    nc.vector.reciprocal_approx_fast(out=rec[:], in_=sacc[:])
                    osb = outs.tile([128, S_HALF], F32, tag="osb")
                    nc.vector.tensor_mul(osb[:], oacc[:], rec[:])
                    nc.sync.dma_start(out=ot[i, :, s0 : s0 + S_HALF], in_=osb[:])

    nc.finalize()
    return nc


def _get_program():
    if "nc" not in _cache:
        _cache["nc"] = _build_program()
    return _cache["nc"]


def kernel(q: np.ndarray, k: np.ndarray, v: np.ndarray) -> np.ndarray:
    from concourse.bass_utils import run_bass_kernel_spmd

    nc = _get_program()

    q4 = np.ascontiguousarray(q, dtype=np.float32).reshape(BH, S, D)
    k4 = np.ascontiguousarray(k, dtype=np.float32).reshape(BH, S, D)
    v4 = np.ascontiguousarray(v, dtype=np.float32).reshape(BH, S, D)

    in_maps = []
    for core in range(N_CORES):
        sl = slice(core * BH_PER_CORE, (core + 1) * BH_PER_CORE)
        in_maps.append(
            {
                "qt": np.ascontiguousarray(q4[sl].transpose(0, 2, 1)),
                "kt": np.ascontiguousarray(k4[sl].transpose(0, 2, 1)),
                "v": np.ascontiguousarray(v4[sl]),
            }
        )

    res = run_bass_kernel_spmd(nc, in_maps, core_ids=list(range(N_CORES)))

    out = np.empty((BH, S, D), dtype=np.float32)
    for core in range(N_CORES):
        ot = res.results[core]["ot"]  # [BH_PER_CORE, D, S]
        out[core * BH_PER_CORE : (core + 1) * BH_PER_CORE] = ot.transpose(0, 2, 1)
    return out.reshape(B, H, S, D)
